# revision 3
# baseline (speedup 1.0000x reference)
"""GAT message-passing kernel for TRN2: host preprocessing + Bass/Tile program.

Design (per core, SPMD over 8 cores, nodes sharded by destination block):
  phase 0: feat = x @ W_gat for own node shard (bf16); el = feat . attn_l,
           er = feat . attn_r; table rows are 512B: [feat(128)|el|1|pad(126)]
           (el and a ones column ride along with every gathered row);
           AllGather the table; er written to er_lin (bf16) in DRAM.
  edge phase (per dst block of 128 nodes, edges pre-sorted by (dst blk, src)):
    - dma_gather the block's edge-source rows (512B rows; <512B DMA penalty
      makes this the same descriptor cost as 256B rows).
    - er_rows [P,128] broadcast-DMA'd from er_lin (dst-block er values
      replicated across partitions).
    - per chunk: er[dst_e] via ONE scalar_tensor_tensor:
        (iota_row == dst_scalar) * er_rows, accum_out -> er_all[:, j].
    - z = el_view + er_all; leaky on DVE; exp on ACT -> alpha fp32 [P, C].
    - per chunk: Oa = tensor_scalar(iota_row, s1=dst, s2=alpha,
        is_equal, mult) -- fused one-hot+alpha in one 2x-mode DVE op.
    - ONE matmul per chunk: lhsT=Oa [e,d], rhs=row[0:130] ([feat|el|1]) ->
      blk_ps[d, 0:130] accumulated over chunks; col 129 = denominator
      (ones column), col 128 = garbage (el), cols 0:128 = agg[d, h].
    - tail: rec = 1/max(den,eps) is PER-PARTITION (no broadcast matmul);
      gene = stt(agg*rec + bias_rows); leaky; PE-transpose; out matmul
      with W_lin^T; per-block DMA to out.
Softmax max-subtraction is dropped (exp args bounded ~ +-16; ratios identical).
"""

import numpy as np
import ml_dtypes
from contextlib import ExitStack

import concourse.bass as bass
import concourse.tile as tile
from concourse import bacc, mybir
from concourse import library_config

dt = mybir.dt
P = 128
ROW = 256          # table row width in bf16 elems (512 bytes)
PAD_DST = 512.0    # one-hot miss sentinel (> 127)


# ---------------------------------------------------------------- host side

def preprocess(src, dst, n_nodes, n_cores):
    """Pure index-space preprocessing (no float math on values)."""
    src = np.asarray(src).astype(np.int64)
    dst = np.asarray(dst).astype(np.int64)
    npc = n_nodes // n_cores                      # nodes per core
    assert npc * n_cores == n_nodes
    blocks = (npc + P - 1) // P
    npc_pad = blocks * P                          # padded nodes per core
    n_pad = npc_pad * n_cores                     # padded global node count
    half = n_pad // 2                             # low table rows [0, half)
    assert half <= 32767 and (n_pad - half) <= 32767
    assert half % npc_pad == 0                    # half boundary between cores

    core_of = dst // npc
    blk_of = (dst % npc) // P
    dloc_of = (dst % npc) % P
    src = (src // npc) * npc_pad + (src % npc)    # padded source coordinates

    lo_lists = [[[] for _ in range(blocks)] for _ in range(n_cores)]
    hi_lists = [[[] for _ in range(blocks)] for _ in range(n_cores)]
    order = np.lexsort((src, blk_of, core_of))
    s_s, c_s, b_s, d_s = src[order], core_of[order], blk_of[order], dloc_of[order]
    hi_mask = s_s >= half
    for c in range(n_cores):
        cm = c_s == c
        for b in range(blocks):
            m = cm & (b_s == b)
            ml = m & ~hi_mask
            mh = m & hi_mask
            lo_lists[c][b] = (s_s[ml], d_s[ml])
            hi_lists[c][b] = (s_s[mh] - half, d_s[mh])

    def nchunks(n):
        return (n + P - 1) // P

    C_lo = [max(max(nchunks(len(lo_lists[c][b][0])) for c in range(n_cores)), 1)
            for b in range(blocks)]
    C_hi = [max(nchunks(len(hi_lists[c][b][0])) for c in range(n_cores))
            for b in range(blocks)]

    total_chunks = sum(C_lo) + sum(C_hi)
    total_L = total_chunks * P

    per_core = []
    for c in range(n_cores):
        idx = np.zeros(total_L, dtype=np.int16)
        dstf = np.full(total_L, PAD_DST, dtype=np.float32)
        off = 0
        for b in range(blocks):
            for lists, C in ((lo_lists, C_lo[b]), (hi_lists, C_hi[b])):
                L = C * P
                if L == 0:
                    continue
                s_arr, d_arr = lists[c][b]
                n = len(s_arr)
                idx[off:off + n] = s_arr.astype(np.int16)
                dstf[off:off + n] = d_arr.astype(np.float32)
                off += L
        assert off == total_L
        idx16 = np.tile(idx.reshape(total_L // 16, 16).T, (8, 1)).copy()
        dstf2 = dstf.reshape(total_chunks, P).T.copy()
        per_core.append({"idx16": idx16, "dstf": dstf2})

    sched = {
        "n_nodes": n_nodes, "n_cores": n_cores, "npc": npc, "blocks": blocks,
        "npc_pad": npc_pad, "n_pad": n_pad,
        "half": half, "C_lo": C_lo, "C_hi": C_hi,
        "total_chunks": total_chunks, "total_L": total_L,
    }
    return sched, per_core


def make_core_inputs(sched, per_core, x, W_gat, attn_l, attn_r, bias_gat, W_lin):
    n_cores, npc, blocks = sched["n_cores"], sched["npc"], sched["blocks"]
    in_f = x.shape[1]
    hid = W_gat.shape[1]
    x = np.asarray(x, dtype=np.float32)
    iota_f = np.broadcast_to(
        np.arange(P, dtype=np.float32), (P, P))
    in_maps = []
    for c in range(n_cores):
        xs = x[c * npc:(c + 1) * npc]
        xpad = np.zeros((blocks * P, in_f), dtype=np.float32)
        xpad[:npc] = xs
        m = {
            "xT": np.ascontiguousarray(xpad.T).astype(ml_dtypes.bfloat16),
            "Wg": np.asarray(W_gat, dtype=np.float32),
            "attnl_b": np.broadcast_to(np.asarray(attn_l, np.float32), (P, hid)).copy(),
            "attnr_b": np.broadcast_to(np.asarray(attn_r, np.float32), (P, hid)).copy(),
            "bias_rows": np.broadcast_to(np.asarray(bias_gat, np.float32), (P, hid)).copy(),
            "WlT": np.ascontiguousarray(np.asarray(W_lin, np.float32).T).astype(
                ml_dtypes.bfloat16),
            "iota_bf": iota_f.astype(ml_dtypes.bfloat16).copy(),
            "iota_colf": np.arange(P, dtype=np.float32)[:, None].copy(),
            "idx16": per_core[c]["idx16"],
            "dstf": per_core[c]["dstf"],
        }
        in_maps.append(m)
    return in_maps


# ---------------------------------------------------------------- device side

def build_program(sched, in_f, hid, out_f, attn_slope=0.2, act_slope=0.01,
                  n_repeat=1, scratch=32768, gmax=6):
    n_cores = sched["n_cores"]
    npc, blocks, half = sched["npc"], sched["blocks"], sched["half"]
    C_lo, C_hi = sched["C_lo"], sched["C_hi"]
    total_chunks, total_L = sched["total_chunks"], sched["total_L"]
    assert in_f % P == 0 and hid == P
    KT = in_f // P

    nc = bacc.Bacc("TRN2", target_bir_lowering=False, debug=False,
                   num_devices=n_cores, dynamic_dma_scratch_size=scratch)

    def din(name, shape, dtype):
        return nc.dram_tensor(name, shape, dtype, kind="ExternalInput").ap()

    xT = din("xT", [in_f, blocks * P], dt.bfloat16)
    Wg = din("Wg", [in_f, hid], dt.float32)
    attnl_b = din("attnl_b", [P, hid], dt.float32)
    attnr_b = din("attnr_b", [P, hid], dt.float32)
    bias_rows = din("bias_rows", [P, hid], dt.float32)
    WlT = din("WlT", [hid, out_f], dt.bfloat16)
    iota_bf = din("iota_bf", [P, P], dt.bfloat16)
    iota_colf = din("iota_colf", [P, 1], dt.float32)
    idx16 = din("idx16", [128, total_L // 16], dt.int16)
    dstf = din("dstf", [P, total_chunks], dt.float32)
    out = nc.dram_tensor("out", [blocks * P, out_f], dt.float32,
                         kind="ExternalOutput").ap()

    tableShard = nc.dram_tensor("tableShard", [blocks * P, ROW],
                                dt.bfloat16).ap()
    tableFull = nc.dram_tensor("tableFull", [sched["n_pad"], ROW],
                               dt.bfloat16, addr_space="Shared").ap()
    er_lin = nc.dram_tensor("er_lin", [blocks * P], dt.bfloat16).ap()

    with ExitStack() as ctx:
        tc = ctx.enter_context(tile.TileContext(nc))
        nc.gpsimd.load_library(library_config.mlp)
        const = ctx.enter_context(tc.tile_pool(name="const", bufs=1))

        iota_sb = const.tile([P, P], dt.bfloat16)
        nc.sync.dma_start(iota_sb[:], iota_bf[:])
        iota_cf = const.tile([P, 1], dt.float32)
        nc.sync.dma_start(iota_cf[:], iota_colf[:])
        attnl_sb = const.tile([P, hid], dt.float32)
        nc.sync.dma_start(attnl_sb[:], attnl_b[:])
        attnr_sb = const.tile([P, hid], dt.float32)
        nc.sync.dma_start(attnr_sb[:], attnr_b[:])
        bias_sb = const.tile([P, hid], dt.float32)
        nc.sync.dma_start(bias_sb[:], bias_rows[:])
        WlT_sb = const.tile([hid, out_f], dt.bfloat16)
        nc.sync.dma_start(WlT_sb[:], WlT[:])
        idx_sb = const.tile([128, total_L // 16], dt.int16)
        nc.sync.dma_start(idx_sb[:], idx16[:])
        dst_sb = const.tile([P, total_chunks], dt.float32)
        nc.sync.dma_start(dst_sb[:], dstf[:])
        # identity (bf16) for PE transpose
        ident = const.tile([P, P], dt.bfloat16)
        nc.vector.tensor_scalar(ident[:], iota_sb[:], iota_cf[:, 0:1], None,
                                mybir.AluOpType.is_equal)

        # ---- phase 0: feat shard + el/er + 512B table rows + AllGather
        ph = ctx.enter_context(tc.tile_pool(name="ph", bufs=1))
        xT_sb = []
        Wg_sb = []
        for k in range(KT):
            t = ph.tile([P, blocks * P], dt.bfloat16, tag=f"xT{k}")
            nc.sync.dma_start(t[:], xT[k * P:(k + 1) * P, :])
            xT_sb.append(t)
            wf = ph.tile([P, hid], dt.float32, tag="Wgf")
            nc.sync.dma_start(wf[:], Wg[k * P:(k + 1) * P, :])
            w = ph.tile([P, hid], dt.bfloat16, tag=f"Wg{k}")
            nc.vector.tensor_copy(w[:], wf[:])
            Wg_sb.append(w)
        er_f = const.tile([P, blocks], dt.float32)
        el_f = const.tile([P, blocks], dt.float32)
        er_bf = const.tile([P, blocks], dt.bfloat16)

        psA = ctx.enter_context(tc.tile_pool(name="psA", bufs=2, space="PSUM"))
        psT = ctx.enter_context(tc.tile_pool(name="psT", bufs=2, space="PSUM"))
        psO = ctx.enter_context(tc.tile_pool(name="psO", bufs=2, space="PSUM"))

        stgp = ctx.enter_context(tc.tile_pool(name="stgp", bufs=3))
        scrp = ctx.enter_context(tc.tile_pool(name="scrp", bufs=3))

        for nb in range(blocks):
            fp = psA.tile([P, hid], dt.float32, tag="psA")
            for k in range(KT):
                nc.tensor.matmul(fp[:, 0:hid],
                                 lhsT=xT_sb[k][:, nb * P:(nb + 1) * P],
                                 rhs=Wg_sb[k][:], start=(k == 0),
                                 stop=(k == KT - 1))
            stg = stgp.tile([P, ROW], dt.bfloat16, tag="stg")
            nc.vector.memset(stg[:, hid + 2:], 0.0)
            nc.vector.memset(stg[:, hid + 1:hid + 2], 1.0)
            nc.vector.tensor_copy(stg[:, 0:hid], fp[:, 0:hid])
            scr = scrp.tile([P, hid], dt.float32, tag="scr")
            nc.vector.scalar_tensor_tensor(
                out=scr[:], in0=fp[:, 0:hid], scalar=1.0, in1=attnr_sb[:],
                op0=mybir.AluOpType.bypass, op1=mybir.AluOpType.mult,
                accum_out=er_f[:, nb:nb + 1])
            scr2 = scrp.tile([P, hid], dt.float32, tag="scr2")
            nc.vector.scalar_tensor_tensor(
                out=scr2[:], in0=fp[:, 0:hid], scalar=1.0, in1=attnl_sb[:],
                op0=mybir.AluOpType.bypass, op1=mybir.AluOpType.mult,
                accum_out=el_f[:, nb:nb + 1])
            nc.vector.tensor_copy(stg[:, hid:hid + 1], el_f[:, nb:nb + 1])
            nc.sync.dma_start(tableShard[nb * P:(nb + 1) * P, :], stg[:])
        nc.vector.tensor_copy(er_bf[:], er_f[:])
        for nb in range(blocks):
            nc.sync.dma_start(er_lin[nb * P:(nb + 1) * P, None],
                              er_bf[:, nb:nb + 1])

        nc.gpsimd.collective_compute(
            "AllGather", mybir.AluOpType.bypass,
            replica_groups=[list(range(n_cores))],
            ins=[tableShard[:].opt()], outs=[tableFull[:].opt()])

        # ---- edge phase
        gp = ctx.enter_context(tc.tile_pool(name="gp", bufs=3))      # G rows
        erp = ctx.enter_context(tc.tile_pool(name="erp", bufs=3))    # er_rows
        oap = ctx.enter_context(tc.tile_pool(name="oap", bufs=4))    # Oa
        jkp = ctx.enter_context(tc.tile_pool(name="jkp", bufs=2))    # junk
        sp = ctx.enter_context(tc.tile_pool(name="sp", bufs=3))      # z/alpha
        tp = ctx.enter_context(tc.tile_pool(name="tp", bufs=3))      # tail
        psB = ctx.enter_context(tc.tile_pool(name="psB", bufs=2, space="PSUM"))

        loop_ctx = tc.For_i(0, n_repeat, 1) if n_repeat > 1 else None
        if loop_ctx is not None:
            loop_ctx.__enter__()
        g = 0
        for b in range(blocks):
            Cl, Ch = C_lo[b], C_hi[b]
            C = Cl + Ch

            # gathered rows [P, C, ROW]
            T = gp.tile([P, C * ROW], dt.bfloat16, tag="T")
            T3 = T[:].rearrange("p (c w) -> p c w", w=ROW)
            o16 = (g * P) // 16
            for cbase, ccnt, tbl in (
                    [(c0, min(gmax, Cl - c0), tableFull[0:half, :])
                     for c0 in range(0, Cl, gmax)] +
                    [(Cl + c0, min(gmax, Ch - c0),
                      tableFull[half:sched["n_pad"], :])
                     for c0 in range(0, Ch, gmax)]):
                nc.gpsimd.dma_gather(
                    T3[:, cbase:cbase + ccnt, :], tbl,
                    idx_sb[:, o16 + cbase * 8:o16 + (cbase + ccnt) * 8],
                    ccnt * P, ccnt * P, ROW, elem_step=ROW)

            # er_rows [P, 128]: dst-block er replicated across partitions
            er_rows = erp.tile([P, P], dt.bfloat16, tag="er_rows")
            nc.sync.dma_start(
                er_rows[:],
                er_lin[b * P:(b + 1) * P][None, :].broadcast_to((P, P)))

            # er_all[:, j] = er[dst_e] via stt accum, one op per chunk
            er_all = sp.tile([P, C], dt.float32, tag="er_all")
            for j in range(C):
                junk = jkp.tile([P, P], dt.bfloat16, tag="junk")
                nc.vector.scalar_tensor_tensor(
                    out=junk[:], in0=iota_sb[:],
                    scalar=dst_sb[:, g + j:g + j + 1], in1=er_rows[:],
                    op0=mybir.AluOpType.is_equal, op1=mybir.AluOpType.mult,
                    accum_out=er_all[:, j:j + 1])

            # z = el + er ; leaky ; alpha = exp  (fp32 [P, C])
            z = sp.tile([P, C], dt.float32, tag="z")
            nc.vector.scalar_tensor_tensor(
                out=z[:], in0=er_all[:], scalar=1.0, in1=T3[:, :, hid],
                op0=mybir.AluOpType.bypass, op1=mybir.AluOpType.add)
            lz = sp.tile([P, C], dt.float32, tag="lz")
            nc.vector.scalar_tensor_tensor(
                out=lz[:], in0=z[:], scalar=float(attn_slope), in1=z[:],
                op0=mybir.AluOpType.mult, op1=mybir.AluOpType.max)
            alpha = sp.tile([P, C], dt.float32, tag="alpha")
            nc.scalar.activation(alpha[:], lz[:],
                                 mybir.ActivationFunctionType.Exp)

            # per chunk: fused one-hot*alpha, then ONE matmul (agg+den)
            blk_ps = psB.tile([P, 130], dt.float32, tag="blk")
            for j in range(C):
                Oa = oap.tile([P, P], dt.bfloat16, tag="Oa")
                nc.vector.tensor_scalar(
                    Oa[:], iota_sb[:], dst_sb[:, g + j:g + j + 1],
                    alpha[:, j:j + 1],
                    mybir.AluOpType.is_equal, mybir.AluOpType.mult)
                nc.tensor.matmul(blk_ps[:], lhsT=Oa[:],
                                 rhs=T3[:, j, 0:hid + 2],
                                 start=(j == 0), stop=(j == C - 1))
            g += C

            # tail: per-partition normalize, bias, leaky, transpose, W_lin
            den = tp.tile([P, 1], dt.float32, tag="den")
            nc.vector.tensor_scalar(den[:], blk_ps[:, 129:130], 1e-30, None,
                                    mybir.AluOpType.max)
            rec = tp.tile([P, 1], dt.float32, tag="rec")
            nc.vector.reciprocal(rec[:], den[:])
            gene = tp.tile([P, hid], dt.float32, tag="gene")
            nc.vector.scalar_tensor_tensor(
                out=gene[:], in0=blk_ps[:, 0:hid], scalar=rec[:, 0:1],
                in1=bias_sb[:], op0=mybir.AluOpType.mult,
                op1=mybir.AluOpType.add)
            geneL = tp.tile([P, hid], dt.bfloat16, tag="geneL")
            nc.vector.scalar_tensor_tensor(
                out=geneL[:], in0=gene[:], scalar=float(act_slope),
                in1=gene[:], op0=mybir.AluOpType.mult, op1=mybir.AluOpType.max)
            gT_ps = psT.tile([P, P], dt.bfloat16, tag="gT")
            nc.tensor.transpose(gT_ps[:], geneL[:], ident[:])
            gT_sb = tp.tile([P, P], dt.bfloat16, tag="gTs")
            nc.scalar.activation(gT_sb[:], gT_ps[:],
                                 mybir.ActivationFunctionType.Copy)
            o_ps = psO.tile([P, out_f], dt.float32, tag="ops")
            nc.tensor.matmul(o_ps[:], lhsT=gT_sb[:], rhs=WlT_sb[:],
                             start=True, stop=True)
            o_sb = tp.tile([P, out_f], dt.float32, tag="osb")
            nc.vector.tensor_copy(o_sb[:], o_ps[:])
            nc.sync.dma_start(out[b * P:(b + 1) * P, :], o_sb[:])
        if loop_ctx is not None:
            loop_ctx.__exit__(None, None, None)

    nc.compile()
    return nc


# ---------------------------------------------------------------- entry point

N_NODES, N_EDGES, IN_F, HID, OUT_F = 50000, 800000, 256, 128, 64
N_CORES = 8

_cache = {}


def kernel(x, src, dst, W_gat, attn_l, attn_r, bias_gat, W_lin):
    """Full-input GAT layer on 8 NeuronCores; returns [N_NODES, OUT_F] fp32."""
    from concourse.bass_utils import run_bass_kernel_spmd

    src = np.asarray(src)
    dst = np.asarray(dst)
    key = (src.tobytes(), dst.tobytes())
    ck = _cache.get("k")
    if ck is not None and ck[0] == key:
        sched, nc = ck[1], ck[2]
    else:
        sched, per_core = preprocess(src, dst, N_NODES, N_CORES)
        _cache["pc"] = per_core
        nc = build_program(sched, IN_F, HID, OUT_F)
        _cache["k"] = (key, sched, nc)
        ck = _cache["k"]
    sched = ck[1]
    per_core = _cache["pc"]
    in_maps = make_core_inputs(sched, per_core, x, W_gat, attn_l, attn_r,
                               bias_gat, W_lin)
    res = run_bass_kernel_spmd(nc, in_maps, core_ids=list(range(N_CORES)))
    out = np.concatenate(
        [res.results[c]["out"][:sched["npc"]] for c in range(N_CORES)], axis=0)
    return out.astype(np.float32)
